# revision 25
# baseline (speedup 1.0000x reference)
"""Trainium2 (8 NeuronCores) kernel for a dense causal multi-head attention block.

Problem shapes: B=2, S=2048, D=2048, H=16, DH=128 (fp32 in/out).

Distribution strategy (sharding_hint: tensor-parallel over heads):
  Phase 1 (head parallel): core c owns heads {2c, 2c+1}. Per (head, batch)
  section it computes Q^T/K^T/V^T = W^T @ X^T in [DH, S] layout, then causal
  attention fully on-chip:
     scores^T[k, q] = K^T.T @ Q^T          (PE, one matmul per 128x512 tile)
     p = exp(scores / sqrt(DH))            (ACT, straight from PSUM)
     diagonal tiles masked by a 0/1 bf16 mask (DVE)
     z^T[dh, q]  += V_tile.T @ p           (PE, PSUM accumulation over k)
     den_bc[128, q] = ones128.T @ p-sums   (PE rank-reduce; the all-ones lhsT
                                            broadcasts den to all partitions)
     z^T *= reciprocal_approx_fast(den_bc) (DVE custom op; no serial [1,512]
                                            divide, no gpsimd broadcast)
  AllToAll (2 MB bf16) per local head reshards z^T from (head-sharded, all
  rows) to (all heads, 512-row shard).
  Phase 2 (row parallel): out[q, d] = Z^T.T @ W_O + b_O for the core's 512
  rows, split by head parity across the two collectives.

Scheduling: the attention inner loop is exp(ACT)-throughput-bound and the PE
executes strictly in order, so every attention stretch drip-feeds independent
matmuls emitted between its own: sections 1-3 interleave the NEXT section's
projection/transpose matmuls (a generator yielding one PE op at a time);
the last section interleaves the even-head half of the output projection.
DMA queues: sync=X^T loads, scalar=weights/biases, vector=z/out stores,
gpsimd=W_O/Z^T phase-2 loads + collectives.

The host wrapper shards/casts inputs (bf16), runs the SPMD NEFF on cores
0-7, and concatenates the per-core row slices into the full output.
"""

import numpy as np
import ml_dtypes

import concourse.bass as bass
import concourse.mybir as mybir
import concourse.tile as tile
from concourse import bacc
from concourse.bass import ts
from concourse.bass_utils import run_bass_kernel_spmd
from concourse.masks import make_identity

B, S, D, H, DH = 2, 2048, 2048, 16, 128
NCORES = 8
HL = H // NCORES            # heads per core = 2
QB = (B * S) // NCORES      # output rows per core = 512
P = 128
SC = 512                    # free-dim chunk (PSUM bank = 512 fp32)
NSC = S // SC               # 4
NDT = D // P                # 16 contraction tiles for D
NST = S // P                # 16 sequence tiles of 128
NQT = QB // P               # 4 local q tiles in phase 2
NDC = D // SC               # 4 output-dim chunks
HP = H // 2                 # heads per parity group = 8
SCALE = 1.0 / float(np.sqrt(DH))
LOOKAHEAD = 3               # scores tiles in flight ahead of z matmuls
QC_END_PULL = 10            # drip items pulled at each q-chunk boundary

F32 = mybir.dt.float32
BF16 = mybir.dt.bfloat16


def build_nc():
    nc = bacc.Bacc("TRN2", target_bir_lowering=False, debug=False,
                   num_devices=NCORES)

    # xt = X^T per batch ([B, D, S]); weights pre-tiled partition-major on the
    # host. wo is parity-grouped on the host: wo[p][j] = W_O rows of head 2j+p.
    xt = nc.dram_tensor("xt", [B, D, S], BF16, kind="ExternalInput")
    wq = nc.dram_tensor("wq", [HL, P, NDT, DH], BF16, kind="ExternalInput")
    wk = nc.dram_tensor("wk", [HL, P, NDT, DH], BF16, kind="ExternalInput")
    wv = nc.dram_tensor("wv", [HL, P, NDT, DH], BF16, kind="ExternalInput")
    bq = nc.dram_tensor("bq", [DH, HL], F32, kind="ExternalInput")
    bk = nc.dram_tensor("bk", [DH, HL], F32, kind="ExternalInput")
    bv = nc.dram_tensor("bv", [DH, HL], F32, kind="ExternalInput")
    wo = nc.dram_tensor("wo", [2, HP, P, D], BF16, kind="ExternalInput")
    bo = nc.dram_tensor("bo", [1, D], BF16, kind="ExternalInput")
    out = nc.dram_tensor("out", [QB, D], F32, kind="ExternalOutput")

    Exp = mybir.ActivationFunctionType.Exp

    with tile.TileContext(nc) as tc:
        with (
            tc.tile_pool(name="const", bufs=1) as cpool,
            tc.tile_pool(name="dram", bufs=1, space="DRAM") as dpool,
            tc.tile_pool(name="ps_acc", bufs=3, space="PSUM") as ps_acc,
            tc.tile_pool(name="ps_p2", bufs=2, space="PSUM") as ps_p2,
            tc.tile_pool(name="ps_z", bufs=2, space="PSUM") as ps_z,
            tc.tile_pool(name="ps_den", bufs=1, space="PSUM") as ps_den,
        ):
            # ---- biases on the scalar queue (sync queue is for X^T) ----
            bias_sb = {}
            for nm, t in (("q", bq), ("k", bk), ("v", bv)):
                bb = cpool.tile([P, HL], F32, tag=f"b{nm}")
                nc.scalar.dma_start(bb, t.ap())
                bias_sb[nm] = bb
            bo_sb = cpool.tile([1, D], BF16)
            nc.scalar.dma_start(bo_sb, bo.ap())

            # one AllToAll per local head index
            a2a_in = [dpool.tile([NCORES, P, SC], BF16, tag=f"a2a_in{hl}",
                                 name=f"a2a_in{hl}") for hl in range(HL)]
            a2a_out = [dpool.tile([NCORES, P, SC], BF16, tag=f"a2a_out{hl}",
                                  name=f"a2a_out{hl}") for hl in range(HL)]

            with (
                tc.tile_pool(name="qkv", bufs=2) as qkvpool,
                tc.tile_pool(name="small", bufs=4) as spool,
                tc.tile_pool(name="xt1", bufs=1) as xtpool_b1,
            ):
              with tc.tile_pool(name="wpool", bufs=1) as wpool:
                # per-head weight tiles [d_part, d_tile, dh]. The very first
                # weight (wq of head 0) rides the sync ring ahead of X^T so
                # the first matmul can start ~9us in; the rest go on the
                # otherwise-idle gpsimd queue.
                w_sb = []
                for hl in range(HL):
                    per = []
                    for nm, w in (("wq", wq), ("wk", wk), ("wv", wv)):
                        t_sb = wpool.tile([P, NDT, DH], BF16, tag=f"{nm}{hl}")
                        eng = nc.sync if (hl == 0 and nm == "wq") \
                            else nc.gpsimd
                        eng.dma_start(t_sb, w.ap()[hl])
                        per.append(t_sb)
                    w_sb.append(per)

                # gpsimd-built constants, emitted after the weight DMA issues
                ident = cpool.tile([P, P], BF16)
                make_identity(nc, ident)
                ones_sq = cpool.tile([P, P], BF16)
                nc.gpsimd.memset(ones_sq, 1.0)
                # mask[ki, t] = 1.0 iff ki <= t: causal triangle, diag tiles
                mask = cpool.tile([P, P], BF16)
                nc.gpsimd.memset(mask, 1.0)
                nc.gpsimd.affine_select(
                    out=mask, in_=mask, compare_op=mybir.AluOpType.is_ge,
                    fill=0.0, base=0, pattern=[[1, P]], channel_multiplier=-1,
                )

                XT = {}
                QKV = {}

                def proj_gen(hl, b):
                    """Generator emitting the (hl, b) projections and V
                    transposes one PE instruction per yield, so they can be
                    dripped into the previous section's attention bubbles."""
                    QT = qkvpool.tile([P, S], BF16, tag="qt")
                    KT = qkvpool.tile([P, S], BF16, tag="kt")
                    VT = qkvpool.tile([P, S], BF16, tag="vt", bufs=1)
                    V_kd = qkvpool.tile([P, NST, DH], BF16, tag="vkd")
                    QKV[hl, b] = (QT, KT, V_kd)
                    for pi, (dst, bcol) in enumerate((
                        (QT, bias_sb["q"]), (KT, bias_sb["k"]),
                        (VT, bias_sb["v"]),
                    )):
                        wt = w_sb[hl][pi]
                        for sc in range(NSC):
                            ps = ps_p2.tile([P, SC], F32, tag="p2")
                            for dt_ in range(NDT):
                                nc.tensor.matmul(
                                    ps, lhsT=wt[:, dt_, :],
                                    rhs=XT[b][:, dt_, ts(sc, SC)],
                                    start=(dt_ == 0), stop=(dt_ == NDT - 1),
                                    skip_group_check=True)
                                yield
                            # drain + bias on DVE (keeps ACT free for exp)
                            nc.vector.tensor_scalar_add(
                                dst[:, ts(sc, SC)], ps, bcol[:, hl:hl + 1])
                    for st in range(NST):
                        pst = ps_p2.tile([P, P], BF16, tag="p2")
                        nc.tensor.matmul(pst, lhsT=VT[:, ts(st, P)],
                                         rhs=ident, is_transpose=True,
                                         skip_group_check=True)
                        nc.vector.tensor_copy(V_kd[:, st, :], pst)
                        yield

                def proj_first(hl, b):
                    """First section's projections: Q/K/V chains interleaved
                    per s-chunk across 3 PSUM banks so each X^T tile is
                    consumed (3 matmuls) as soon as its DMA lands — the PE
                    tracks the arrival wave instead of stalling per chain."""
                    QT = qkvpool.tile([P, S], BF16, tag="qt")
                    KT = qkvpool.tile([P, S], BF16, tag="kt")
                    VT = qkvpool.tile([P, S], BF16, tag="vt", bufs=1)
                    V_kd = qkvpool.tile([P, NST, DH], BF16, tag="vkd")
                    QKV[hl, b] = (QT, KT, V_kd)
                    dsts = ((QT, bias_sb["q"]), (KT, bias_sb["k"]),
                            (VT, bias_sb["v"]))
                    for sc in range(NSC):
                        banks = [ps_acc.tile([P, SC], F32, tag="acc",
                                             name=f"pf{sc}_{i}")
                                 for i in range(3)]
                        for dt_ in range(NDT):
                            for pi in range(3):
                                nc.tensor.matmul(
                                    banks[pi], lhsT=w_sb[hl][pi][:, dt_, :],
                                    rhs=XT[b][:, dt_, ts(sc, SC)],
                                    start=(dt_ == 0), stop=(dt_ == NDT - 1),
                                    skip_group_check=True)
                        for pi, (dst, bcol) in enumerate(dsts):
                            nc.vector.tensor_scalar_add(
                                dst[:, ts(sc, SC)], banks[pi],
                                bcol[:, hl:hl + 1])
                    for st in range(NST):
                        pst = ps_p2.tile([P, P], BF16, tag="p2")
                        nc.tensor.matmul(pst, lhsT=VT[:, ts(st, P)],
                                         rhs=ident, is_transpose=True,
                                         skip_group_check=True)
                        nc.vector.tensor_copy(V_kd[:, st, :], pst)

                def drain(g):
                    for _ in g:
                        pass

                def pull(g, n):
                    if g is None:
                        return
                    for _ in range(n):
                        if next(g, StopIteration) is StopIteration:
                            return

                def attention(hl, b, drip=None, drip_from=0):
                    """Causal attention for (hl, b); scores pipelined
                    LOOKAHEAD tiles ahead; diagonal tiles at reduced width.
                    One drip item is pulled per kt step, QC_END_PULL per
                    q-chunk boundary, filling PE bubbles left by exp."""
                    QT, KT, V_kd = QKV[hl, b]
                    for qc in range(NSC):
                        dripping = drip if qc >= drip_from else None
                        z_ps = ps_z.tile([P, SC], F32, tag="z")
                        nkt = 4 * qc + 4
                        pexps = {}
                        # exp-sum accumulators: four short bf16 chains keep
                        # the DVE in 2x mode and off the critical path
                        dacc = [spool.tile([P, SC], BF16, tag=f"dac{c}",
                                           bufs=1, name=f"dac{c}")
                                for c in range(4)]

                        def emit_scores(kt, qc=qc, pexps=None, dacc=dacc):
                            j = kt - 4 * qc
                            lo = 128 * j if j >= 0 else 0
                            s_ps = ps_acc.tile([P, SC], F32, tag="acc")
                            nc.tensor.matmul(
                                s_ps[:, :SC - lo], lhsT=KT[:, ts(kt, P)],
                                rhs=QT[:, qc * SC + lo:(qc + 1) * SC],
                                start=True, stop=True)
                            pexp = spool.tile([P, SC], BF16, tag="p", bufs=5)
                            nc.scalar.activation(
                                pexp[:, lo:], s_ps[:, :SC - lo], Exp,
                                bias=0.0, scale=SCALE)
                            if j >= 0:
                                nc.vector.tensor_mul(
                                    pexp[:, lo:lo + P], pexp[:, lo:lo + P],
                                    mask)
                            da = dacc[kt % 4]
                            if kt < 4:
                                nc.vector.tensor_copy(da[:, lo:], pexp[:, lo:])
                            else:
                                nc.vector.tensor_add(
                                    da[:, lo:], da[:, lo:], pexp[:, lo:])
                            pexps[kt] = (pexp, lo)

                        def emit_den(qc=qc, dacc=dacc):
                            # merge chains pairwise on DVE, then one all-ones
                            # matmul both reduces over k AND broadcasts den
                            # across all 128 partitions.
                            clo = [128 * c if qc == 0 else 0 for c in range(4)]
                            nc.vector.tensor_add(
                                dacc[0][:, clo[1]:], dacc[0][:, clo[1]:],
                                dacc[1][:, clo[1]:])
                            nc.vector.tensor_add(
                                dacc[2][:, clo[3]:], dacc[2][:, clo[3]:],
                                dacc[3][:, clo[3]:])
                            nc.vector.tensor_add(
                                dacc[0][:, clo[2]:], dacc[0][:, clo[2]:],
                                dacc[2][:, clo[2]:])
                            den_bc = ps_den.tile([P, SC], F32, tag="den")
                            nc.tensor.matmul(den_bc, lhsT=ones_sq,
                                             rhs=dacc[0], start=True,
                                             stop=True)
                            rb = spool.tile([P, SC], F32, tag="rb", bufs=2)
                            nc.vector.reciprocal_approx_fast(out=rb,
                                                             in_=den_bc)
                            return rb

                        for k0 in range(min(LOOKAHEAD, nkt)):
                            emit_scores(k0, pexps=pexps)
                        rb = None
                        if nkt <= LOOKAHEAD:
                            rb = emit_den()
                        for kt in range(nkt):
                            pull(dripping, 2)
                            if kt + LOOKAHEAD < nkt:
                                emit_scores(kt + LOOKAHEAD, pexps=pexps)
                                if kt + LOOKAHEAD == nkt - 1:
                                    rb = emit_den()
                            pexp, lo = pexps.pop(kt)
                            nc.tensor.matmul(
                                z_ps[:, lo:], lhsT=V_kd[:, kt, :],
                                rhs=pexp[:, lo:],
                                start=(kt == 0), stop=(kt == nkt - 1),
                                skip_group_check=True)
                        zs = spool.tile([P, SC], BF16, tag="zs", bufs=2)
                        nc.vector.tensor_mul(zs, z_ps, rb)
                        nc.scalar.dma_start(a2a_in[hl][4 * b + qc], zs)
                        pull(dripping, QC_END_PULL)

                # ---------- phase-2 helpers ----------
                p2state = {}

                def p2_open(p2pool):
                    # per-parity Z^T tiles: keeps the odd-half DMA writes
                    # (gated on the 2nd collective) from falsely blocking
                    # even-half reads
                    ZTs = [p2pool.tile([P, HP, SC], BF16, tag=f"zt{par}",
                                       name=f"zt{par}") for par in range(2)]
                    bo_b = p2pool.tile([P, D], BF16, tag="bo_b")
                    nc.gpsimd.partition_broadcast(bo_b, bo_sb)
                    parts = {}
                    for qt in range(NQT):
                        for dc in range(NDC):
                            parts[qt, dc] = p2pool.tile(
                                [P, SC], BF16, tag=f"part{qt}_{dc}",
                                name=f"part{qt}_{dc}")
                    p2state.update(ZTs=ZTs, bo_b=bo_b, parts=parts,
                                   pool=p2pool)
                    p2_load_chunk(0, 0, nc.gpsimd)
                    for j in range(NCORES):
                        nc.gpsimd.dma_start(ZTs[0][:, j, :], a2a_out[0][j])

                def p2_load_chunk(par, dc, eng=None):
                    """Stream one 512-col W_O chunk of a parity group;
                    double-buffered per parity so the next chunk prefetches
                    under the current slots."""
                    WOc = p2state["pool"].tile([P, HP, SC], BF16,
                                               tag=f"woc{par}", bufs=2,
                                               name=f"woc{par}_{dc}")
                    p2state["WOc", par, dc] = WOc
                    eng = eng or nc.gpsimd
                    for j in range(HP):
                        eng.dma_start(WOc[:, j, :],
                                      wo.ap()[par][j][:, ts(dc, SC)])

                def p2_slot(par, dc, qt):
                    """Accumulate 8 parity heads into the (qt, dc) output
                    tile; yields per head."""
                    ZT, WOc = p2state["ZTs"][par], p2state["WOc", par, dc]
                    pa = ps_p2.tile([P, SC], F32, tag="p2")
                    for j in range(HP):
                        nc.tensor.matmul(pa, lhsT=ZT[:, j, ts(qt, P)],
                                         rhs=WOc[:, j, :],
                                         start=(j == 0), stop=(j == HP - 1),
                                         skip_group_check=True)
                        yield
                    if par == 0:
                        nc.vector.tensor_add(
                            p2state["parts"][qt, dc], pa,
                            p2state["bo_b"][:, ts(dc, SC)])
                    else:
                        osb = p2state["pool"].tile([P, SC], F32,
                                                   tag="osb", bufs=2)
                        nc.vector.tensor_add(osb, pa,
                                             p2state["parts"][qt, dc])
                        nc.scalar.dma_start(
                            out.ap()[ts(qt, P), ts(dc, SC)], osb)

                def p2half_gen(par):
                    for dc in range(NDC):
                        if dc + 1 < NDC:
                            p2_load_chunk(par, dc + 1)
                        for qt in range(NQT):
                            yield from p2_slot(par, dc, qt)

                # ---------- phase 1 ----------
                with tc.tile_pool(name="xt0", bufs=1) as xtpool_b0:
                    for b in range(B):
                        pool = xtpool_b0 if b == 0 else xtpool_b1
                        xtt = pool.tile([P, NDT, S], BF16, tag=f"xt{b}",
                                        name=f"xt{b}")
                        if b == 0:
                            # s-chunk-major so the first projection chain is
                            # paced by 128KB slices, not 512KB rows
                            for sc in range(NSC):
                                for dt_ in range(NDT):
                                    nc.sync.dma_start(
                                        xtt[:, dt_, ts(sc, SC)],
                                        xt.ap()[b][ts(dt_, P), ts(sc, SC)])
                        else:
                            for dt_ in range(NDT):
                                nc.sync.dma_start(xtt[:, dt_, :],
                                                  xt.ap()[b][ts(dt_, P), :])
                        XT[b] = xtt

                    proj_first(0, 0)
                    g01 = proj_gen(0, 1)
                    attention(0, 0, drip=g01)
                    drain(g01)
                    g10 = proj_gen(1, 0)
                    attention(0, 1, drip=g10)
                    nc.gpsimd.collective_compute(
                        "AllToAll", mybir.AluOpType.bypass,
                        replica_groups=[list(range(NCORES))],
                        ins=[a2a_in[0][:]], outs=[a2a_out[0][:]],
                    )
                    drain(g10)
                    g11 = proj_gen(1, 1)
                    attention(1, 0, drip=g11)
                drain(g11)
              # wpool + xtpool_b0 closed: their SBUF feeds phase-2 tiles
              with tc.tile_pool(name="p2", bufs=1) as p2pool:
                p2_open(p2pool)
                attention(1, 1)
                nc.gpsimd.collective_compute(
                    "AllToAll", mybir.AluOpType.bypass,
                    replica_groups=[list(range(NCORES))],
                    ins=[a2a_in[1][:]], outs=[a2a_out[1][:]],
                )
                # odd-head Z^T loads on the (now idle) scalar ring so
                # they don't queue behind W_O chunk issues on gpsimd
                for j in range(NCORES):
                    nc.scalar.dma_start(p2state["ZTs"][1][:, j, :],
                                        a2a_out[1][j])
                # the whole even-head half runs during the collective,
                # absorbing peer launch skew at this sync point
                drain(p2half_gen(0))
                p2_load_chunk(1, 0)
                drain(p2half_gen(1))

    nc.compile()
    return nc


_CACHE = {}


def _get_nc():
    if "nc" not in _CACHE:
        _CACHE["nc"] = build_nc()
    return _CACHE["nc"]


def make_in_maps(resid_pre, W_Q, W_K, W_V, W_O, b_Q, b_K, b_V, b_O):
    bf = ml_dtypes.bfloat16
    x_bf = np.asarray(resid_pre, np.float32).astype(bf)
    xt = np.ascontiguousarray(x_bf.transpose(0, 2, 1))  # [B, D, S]
    # weights pre-tiled to [H, P, NDT, DH]: w_t[h, p, o, k] = W[h, o*P + p, k]
    def tile_w(W):
        Wb = np.asarray(W, np.float32).astype(bf)
        return np.ascontiguousarray(
            Wb.reshape(H, NDT, P, DH).transpose(0, 2, 1, 3))
    WQ, WK, WV = tile_w(W_Q), tile_w(W_K), tile_w(W_V)
    # wo parity-grouped: wo[p][j] = W_O rows of head 2j+p -> [2, HP, DH, D]
    WOr = np.asarray(W_O, np.float32).reshape(H, DH, D)
    WOp = np.ascontiguousarray(np.stack([WOr[0::2], WOr[1::2]])).astype(bf)
    bQ = np.ascontiguousarray(np.asarray(b_Q, np.float32).T)  # [DH, H]
    bK = np.ascontiguousarray(np.asarray(b_K, np.float32).T)
    bV = np.ascontiguousarray(np.asarray(b_V, np.float32).T)
    bO = np.ascontiguousarray(
        np.asarray(b_O, np.float32)).reshape(1, D).astype(bf)
    in_maps = []
    for c in range(NCORES):
        hs = slice(c * HL, (c + 1) * HL)
        in_maps.append({
            "xt": xt,
            "wq": np.ascontiguousarray(WQ[hs]),
            "wk": np.ascontiguousarray(WK[hs]),
            "wv": np.ascontiguousarray(WV[hs]),
            "bq": np.ascontiguousarray(bQ[:, hs]),
            "bk": np.ascontiguousarray(bK[:, hs]),
            "bv": np.ascontiguousarray(bV[:, hs]),
            "wo": WOp,
            "bo": bO,
        })
    return in_maps


def assemble(results):
    out = np.empty((B, S, D), np.float32)
    for c in range(NCORES):
        b, r = divmod(c, NCORES // B)  # divmod(c, 4)
        out[b, r * QB:(r + 1) * QB] = results[c]["out"]
    return out


def kernel(resid_pre, W_Q, W_K, W_V, W_O, b_Q, b_K, b_V, b_O,
           _trace=False, _return_raw=False):
    nc = _get_nc()
    in_maps = make_in_maps(resid_pre, W_Q, W_K, W_V, W_O, b_Q, b_K, b_V, b_O)
    res = run_bass_kernel_spmd(nc, in_maps, core_ids=list(range(NCORES)),
                               trace=_trace)
    out = assemble(res.results)
    if _return_raw:
        return out, res
    return out


# revision 26
# speedup vs baseline: 1.0169x; 1.0169x over previous
"""Trainium2 (8 NeuronCores) kernel for a dense causal multi-head attention block.

Problem shapes: B=2, S=2048, D=2048, H=16, DH=128 (fp32 in/out).

Distribution strategy (sharding_hint: tensor-parallel over heads):
  Phase 1 (head parallel): core c owns heads {2c, 2c+1}. Per (head, batch)
  section it computes Q^T/K^T/V^T = W^T @ X^T in [DH, S] layout, then causal
  attention fully on-chip:
     scores^T[k, q] = K^T.T @ Q^T          (PE, one matmul per 128x512 tile)
     p = exp(scores / sqrt(DH))            (ACT, straight from PSUM)
     diagonal tiles masked by a 0/1 bf16 mask (DVE)
     z^T[dh, q]  += V_tile.T @ p           (PE, PSUM accumulation over k)
     den_bc[128, q] = ones128.T @ p-sums   (PE rank-reduce; the all-ones lhsT
                                            broadcasts den to all partitions)
     z^T *= reciprocal_approx_fast(den_bc) (DVE custom op; no serial [1,512]
                                            divide, no gpsimd broadcast)
  AllToAll (2 MB bf16) per local head reshards z^T from (head-sharded, all
  rows) to (all heads, 512-row shard).
  Phase 2 (row parallel): out[q, d] = Z^T.T @ W_O + b_O for the core's 512
  rows, split by head parity across the two collectives.

Scheduling: the attention inner loop is exp(ACT)-throughput-bound and the PE
executes strictly in order, so every attention stretch drip-feeds independent
matmuls emitted between its own: sections 1-3 interleave the NEXT section's
projection/transpose matmuls (a generator yielding one PE op at a time);
the last section interleaves the even-head half of the output projection.
DMA queues: sync=X^T loads, scalar=weights/biases, vector=z/out stores,
gpsimd=W_O/Z^T phase-2 loads + collectives.

The host wrapper shards/casts inputs (bf16), runs the SPMD NEFF on cores
0-7, and concatenates the per-core row slices into the full output.
"""

import numpy as np
import ml_dtypes

import concourse.bass as bass
import concourse.mybir as mybir
import concourse.tile as tile
from concourse import bacc
from concourse.bass import ts
from concourse.bass_utils import run_bass_kernel_spmd
from concourse.masks import make_identity

B, S, D, H, DH = 2, 2048, 2048, 16, 128
NCORES = 8
HL = H // NCORES            # heads per core = 2
QB = (B * S) // NCORES      # output rows per core = 512
P = 128
SC = 512                    # free-dim chunk (PSUM bank = 512 fp32)
NSC = S // SC               # 4
NDT = D // P                # 16 contraction tiles for D
NST = S // P                # 16 sequence tiles of 128
NQT = QB // P               # 4 local q tiles in phase 2
NDC = D // SC               # 4 output-dim chunks
HP = H // 2                 # heads per parity group = 8
SCALE = 1.0 / float(np.sqrt(DH))
LOOKAHEAD = 3               # scores tiles in flight ahead of z matmuls
QC_END_PULL = 10            # drip items pulled at each q-chunk boundary

F32 = mybir.dt.float32
BF16 = mybir.dt.bfloat16


def build_nc():
    nc = bacc.Bacc("TRN2", target_bir_lowering=False, debug=False,
                   num_devices=NCORES)

    # xt = X^T per batch ([B, D, S]); weights pre-tiled partition-major on the
    # host. wo is parity-grouped on the host: wo[p][j] = W_O rows of head 2j+p.
    xt = nc.dram_tensor("xt", [B, D, S], BF16, kind="ExternalInput")
    wq = nc.dram_tensor("wq", [HL, P, NDT, DH], BF16, kind="ExternalInput")
    wk = nc.dram_tensor("wk", [HL, P, NDT, DH], BF16, kind="ExternalInput")
    wv = nc.dram_tensor("wv", [HL, P, NDT, DH], BF16, kind="ExternalInput")
    bq = nc.dram_tensor("bq", [DH, HL], F32, kind="ExternalInput")
    bk = nc.dram_tensor("bk", [DH, HL], F32, kind="ExternalInput")
    bv = nc.dram_tensor("bv", [DH, HL], F32, kind="ExternalInput")
    wo = nc.dram_tensor("wo", [2, HP, P, D], BF16, kind="ExternalInput")
    bo = nc.dram_tensor("bo", [1, D], BF16, kind="ExternalInput")
    out = nc.dram_tensor("out", [QB, D], F32, kind="ExternalOutput")

    Exp = mybir.ActivationFunctionType.Exp

    with tile.TileContext(nc) as tc:
        with (
            tc.tile_pool(name="const", bufs=1) as cpool,
            tc.tile_pool(name="dram", bufs=1, space="DRAM") as dpool,
            tc.tile_pool(name="ps_acc", bufs=3, space="PSUM") as ps_acc,
            tc.tile_pool(name="ps_p2", bufs=2, space="PSUM") as ps_p2,
            tc.tile_pool(name="ps_z", bufs=2, space="PSUM") as ps_z,
            tc.tile_pool(name="ps_den", bufs=1, space="PSUM") as ps_den,
        ):
            # ---- biases on the scalar queue (sync queue is for X^T) ----
            bias_sb = {}
            for nm, t in (("q", bq), ("k", bk), ("v", bv)):
                bb = cpool.tile([P, HL], F32, tag=f"b{nm}")
                nc.scalar.dma_start(bb, t.ap())
                bias_sb[nm] = bb
            bo_sb = cpool.tile([1, D], BF16)
            nc.scalar.dma_start(bo_sb, bo.ap())

            # one AllToAll per local head index
            a2a_in = [dpool.tile([NCORES, P, SC], BF16, tag=f"a2a_in{hl}",
                                 name=f"a2a_in{hl}") for hl in range(HL)]
            a2a_out = [dpool.tile([NCORES, P, SC], BF16, tag=f"a2a_out{hl}",
                                  name=f"a2a_out{hl}") for hl in range(HL)]

            with (
                tc.tile_pool(name="qkv", bufs=2) as qkvpool,
                tc.tile_pool(name="small", bufs=4) as spool,
                tc.tile_pool(name="xt1", bufs=1) as xtpool_b1,
            ):
              with tc.tile_pool(name="wpool", bufs=1) as wpool:
                # per-head weight tiles [d_part, d_tile, dh]. The very first
                # weight (wq of head 0) rides the sync ring ahead of X^T so
                # the first matmul can start ~9us in; the rest go on the
                # otherwise-idle gpsimd queue.
                w_sb = []
                for hl in range(HL):
                    per = []
                    for nm, w in (("wq", wq), ("wk", wk), ("wv", wv)):
                        t_sb = wpool.tile([P, NDT, DH], BF16, tag=f"{nm}{hl}")
                        eng = nc.sync if (hl == 0 and nm == "wq") \
                            else nc.gpsimd
                        eng.dma_start(t_sb, w.ap()[hl])
                        per.append(t_sb)
                    w_sb.append(per)

                # gpsimd-built constants, emitted after the weight DMA issues
                ident = cpool.tile([P, P], BF16)
                make_identity(nc, ident)
                ones_sq = cpool.tile([P, P], BF16)
                nc.gpsimd.memset(ones_sq, 1.0)
                # mask[ki, t] = 1.0 iff ki <= t: causal triangle, diag tiles
                mask = cpool.tile([P, P], BF16)
                nc.gpsimd.memset(mask, 1.0)
                nc.gpsimd.affine_select(
                    out=mask, in_=mask, compare_op=mybir.AluOpType.is_ge,
                    fill=0.0, base=0, pattern=[[1, P]], channel_multiplier=-1,
                )

                XT = {}
                QKV = {}

                def proj_gen(hl, b):
                    """Generator emitting the (hl, b) projections and V
                    transposes one PE instruction per yield, so they can be
                    dripped into the previous section's attention bubbles."""
                    QT = qkvpool.tile([P, S], BF16, tag="qt")
                    KT = qkvpool.tile([P, S], BF16, tag="kt")
                    VT = qkvpool.tile([P, S], BF16, tag="vt", bufs=1)
                    V_kd = qkvpool.tile([P, NST, DH], BF16, tag="vkd")
                    QKV[hl, b] = (QT, KT, V_kd)
                    for pi, (dst, bcol) in enumerate((
                        (QT, bias_sb["q"]), (KT, bias_sb["k"]),
                        (VT, bias_sb["v"]),
                    )):
                        wt = w_sb[hl][pi]
                        for sc in range(NSC):
                            ps = ps_p2.tile([P, SC], F32, tag="p2")
                            for dt_ in range(NDT):
                                nc.tensor.matmul(
                                    ps, lhsT=wt[:, dt_, :],
                                    rhs=XT[b][:, dt_, ts(sc, SC)],
                                    start=(dt_ == 0), stop=(dt_ == NDT - 1),
                                    skip_group_check=True)
                                yield
                            # drain + bias on DVE (keeps ACT free for exp)
                            nc.vector.tensor_scalar_add(
                                dst[:, ts(sc, SC)], ps, bcol[:, hl:hl + 1])
                    for st in range(NST):
                        pst = ps_p2.tile([P, P], BF16, tag="p2")
                        nc.tensor.matmul(pst, lhsT=VT[:, ts(st, P)],
                                         rhs=ident, is_transpose=True,
                                         skip_group_check=True)
                        nc.vector.tensor_copy(V_kd[:, st, :], pst)
                        yield

                def proj_first(hl, b):
                    """First section's projections: Q/K/V chains interleaved
                    per s-chunk across 3 PSUM banks so each X^T tile is
                    consumed (3 matmuls) as soon as its DMA lands — the PE
                    tracks the arrival wave instead of stalling per chain."""
                    QT = qkvpool.tile([P, S], BF16, tag="qt")
                    KT = qkvpool.tile([P, S], BF16, tag="kt")
                    VT = qkvpool.tile([P, S], BF16, tag="vt", bufs=1)
                    V_kd = qkvpool.tile([P, NST, DH], BF16, tag="vkd")
                    QKV[hl, b] = (QT, KT, V_kd)
                    dsts = ((QT, bias_sb["q"]), (KT, bias_sb["k"]),
                            (VT, bias_sb["v"]))
                    for sc in range(NSC):
                        banks = [ps_acc.tile([P, SC], F32, tag="acc",
                                             name=f"pf{sc}_{i}")
                                 for i in range(3)]
                        for dt_ in range(NDT):
                            for pi in range(3):
                                nc.tensor.matmul(
                                    banks[pi], lhsT=w_sb[hl][pi][:, dt_, :],
                                    rhs=XT[b][:, dt_, ts(sc, SC)],
                                    start=(dt_ == 0), stop=(dt_ == NDT - 1),
                                    skip_group_check=True)
                        for pi, (dst, bcol) in enumerate(dsts):
                            nc.vector.tensor_scalar_add(
                                dst[:, ts(sc, SC)], banks[pi],
                                bcol[:, hl:hl + 1])
                    for st in range(NST):
                        pst = ps_p2.tile([P, P], BF16, tag="p2")
                        nc.tensor.matmul(pst, lhsT=VT[:, ts(st, P)],
                                         rhs=ident, is_transpose=True,
                                         skip_group_check=True)
                        nc.vector.tensor_copy(V_kd[:, st, :], pst)

                def drain(g):
                    for _ in g:
                        pass

                def pull(g, n):
                    if g is None:
                        return
                    for _ in range(n):
                        if next(g, StopIteration) is StopIteration:
                            return

                def attention(hl, b, drip=None, drip_from=0):
                    """Causal attention for (hl, b); scores pipelined
                    LOOKAHEAD tiles ahead; diagonal tiles at reduced width.
                    One drip item is pulled per kt step, QC_END_PULL per
                    q-chunk boundary, filling PE bubbles left by exp."""
                    QT, KT, V_kd = QKV[hl, b]
                    for qc in range(NSC):
                        dripping = drip if qc >= drip_from else None
                        z_ps = ps_z.tile([P, SC], F32, tag="z")
                        nkt = 4 * qc + 4
                        pexps = {}
                        # exp-sum accumulators: four short bf16 chains keep
                        # the DVE in 2x mode and off the critical path
                        dacc = [spool.tile([P, SC], BF16, tag=f"dac{c}",
                                           bufs=1, name=f"dac{c}")
                                for c in range(4)]

                        def emit_scores(kt, qc=qc, pexps=None, dacc=dacc):
                            j = kt - 4 * qc
                            lo = 128 * j if j >= 0 else 0
                            s_ps = ps_acc.tile([P, SC], F32, tag="acc")
                            nc.tensor.matmul(
                                s_ps[:, :SC - lo], lhsT=KT[:, ts(kt, P)],
                                rhs=QT[:, qc * SC + lo:(qc + 1) * SC],
                                start=True, stop=True)
                            pexp = spool.tile([P, SC], BF16, tag="p", bufs=5)
                            nc.scalar.activation(
                                pexp[:, lo:], s_ps[:, :SC - lo], Exp,
                                bias=0.0, scale=SCALE)
                            if j >= 0:
                                nc.vector.tensor_mul(
                                    pexp[:, lo:lo + P], pexp[:, lo:lo + P],
                                    mask)
                            da = dacc[kt % 4]
                            if kt < 4:
                                nc.vector.tensor_copy(da[:, lo:], pexp[:, lo:])
                            else:
                                nc.vector.tensor_add(
                                    da[:, lo:], da[:, lo:], pexp[:, lo:])
                            pexps[kt] = (pexp, lo)

                        def emit_den(qc=qc, dacc=dacc):
                            # merge chains pairwise on DVE, then one all-ones
                            # matmul both reduces over k AND broadcasts den
                            # across all 128 partitions.
                            clo = [128 * c if qc == 0 else 0 for c in range(4)]
                            nc.vector.tensor_add(
                                dacc[0][:, clo[1]:], dacc[0][:, clo[1]:],
                                dacc[1][:, clo[1]:])
                            nc.vector.tensor_add(
                                dacc[2][:, clo[3]:], dacc[2][:, clo[3]:],
                                dacc[3][:, clo[3]:])
                            nc.vector.tensor_add(
                                dacc[0][:, clo[2]:], dacc[0][:, clo[2]:],
                                dacc[2][:, clo[2]:])
                            den_bc = ps_den.tile([P, SC], F32, tag="den")
                            nc.tensor.matmul(den_bc, lhsT=ones_sq,
                                             rhs=dacc[0], start=True,
                                             stop=True)
                            rb = spool.tile([P, SC], F32, tag="rb", bufs=2)
                            nc.vector.reciprocal_approx_fast(out=rb,
                                                             in_=den_bc)
                            return rb

                        for k0 in range(min(LOOKAHEAD, nkt)):
                            emit_scores(k0, pexps=pexps)
                        rb = None
                        if nkt <= LOOKAHEAD:
                            rb = emit_den()
                        for kt in range(nkt):
                            pull(dripping, 2)
                            if kt + LOOKAHEAD < nkt:
                                emit_scores(kt + LOOKAHEAD, pexps=pexps)
                                if kt + LOOKAHEAD == nkt - 1:
                                    rb = emit_den()
                            pexp, lo = pexps.pop(kt)
                            nc.tensor.matmul(
                                z_ps[:, lo:], lhsT=V_kd[:, kt, :],
                                rhs=pexp[:, lo:],
                                start=(kt == 0), stop=(kt == nkt - 1),
                                skip_group_check=True)
                        zs = spool.tile([P, SC], BF16, tag="zs", bufs=2)
                        nc.vector.tensor_mul(zs, z_ps, rb)
                        nc.scalar.dma_start(a2a_in[hl][4 * b + qc], zs)
                        pull(dripping, QC_END_PULL)

                # ---------- phase-2 helpers ----------
                p2state = {}

                def p2_open(p2pool):
                    # per-parity Z^T tiles: keeps the odd-half DMA writes
                    # (gated on the 2nd collective) from falsely blocking
                    # even-half reads
                    ZTs = [p2pool.tile([P, HP, SC], BF16, tag=f"zt{par}",
                                       name=f"zt{par}") for par in range(2)]
                    bo_b = p2pool.tile([P, D], BF16, tag="bo_b")
                    nc.gpsimd.partition_broadcast(bo_b, bo_sb)
                    parts = {}
                    for qt in range(NQT):
                        for dc in range(NDC):
                            parts[qt, dc] = p2pool.tile(
                                [P, SC], BF16, tag=f"part{qt}_{dc}",
                                name=f"part{qt}_{dc}")
                    p2state.update(ZTs=ZTs, bo_b=bo_b, parts=parts,
                                   pool=p2pool)
                    p2_load_chunk(0, 0, nc.gpsimd)
                    for j in range(NCORES):
                        nc.gpsimd.dma_start(ZTs[0][:, j, :], a2a_out[0][j])

                def p2_load_chunk(par, dc, eng=None):
                    """Stream one 512-col W_O chunk of a parity group;
                    double-buffered per parity so the next chunk prefetches
                    under the current slots."""
                    WOc = p2state["pool"].tile([P, HP, SC], BF16,
                                               tag=f"woc{par}", bufs=2,
                                               name=f"woc{par}_{dc}")
                    p2state["WOc", par, dc] = WOc
                    eng = eng or nc.gpsimd
                    for j in range(HP):
                        eng.dma_start(WOc[:, j, :],
                                      wo.ap()[par][j][:, ts(dc, SC)])

                def p2_slot(par, dc, qt):
                    """Accumulate 8 parity heads into the (qt, dc) output
                    tile; yields per head."""
                    ZT, WOc = p2state["ZTs"][par], p2state["WOc", par, dc]
                    pa = ps_p2.tile([P, SC], F32, tag="p2")
                    for j in range(HP):
                        nc.tensor.matmul(pa, lhsT=ZT[:, j, ts(qt, P)],
                                         rhs=WOc[:, j, :],
                                         start=(j == 0), stop=(j == HP - 1),
                                         skip_group_check=True)
                        yield
                    if par == 0:
                        nc.vector.tensor_add(
                            p2state["parts"][qt, dc], pa,
                            p2state["bo_b"][:, ts(dc, SC)])
                    else:
                        osb = p2state["pool"].tile([P, SC], F32,
                                                   tag="osb", bufs=2)
                        nc.vector.tensor_add(osb, pa,
                                             p2state["parts"][qt, dc])
                        nc.scalar.dma_start(
                            out.ap()[ts(qt, P), ts(dc, SC)], osb)

                def p2half_gen(par):
                    for dc in range(NDC):
                        if dc + 1 < NDC:
                            p2_load_chunk(par, dc + 1)
                        for qt in range(NQT):
                            yield from p2_slot(par, dc, qt)

                # ---------- phase 1 ----------
                with tc.tile_pool(name="xt0", bufs=1) as xtpool_b0:
                    for b in range(B):
                        pool = xtpool_b0 if b == 0 else xtpool_b1
                        xtt = pool.tile([P, NDT, S], BF16, tag=f"xt{b}",
                                        name=f"xt{b}")
                        if b == 0:
                            # s-chunk-major so the first projection chain is
                            # paced by 128KB slices, not 512KB rows
                            for sc in range(NSC):
                                for dt_ in range(NDT):
                                    nc.sync.dma_start(
                                        xtt[:, dt_, ts(sc, SC)],
                                        xt.ap()[b][ts(dt_, P), ts(sc, SC)])
                        else:
                            for dt_ in range(NDT):
                                nc.sync.dma_start(xtt[:, dt_, :],
                                                  xt.ap()[b][ts(dt_, P), :])
                        XT[b] = xtt

                    proj_first(0, 0)
                    g01 = proj_gen(0, 1)
                    attention(0, 0, drip=g01)
                    drain(g01)
                    g10 = proj_gen(1, 0)
                    attention(0, 1, drip=g10)
                    nc.gpsimd.collective_compute(
                        "AllToAll", mybir.AluOpType.bypass,
                        replica_groups=[list(range(NCORES))],
                        ins=[a2a_in[0][:]], outs=[a2a_out[0][:]],
                    )
                    drain(g10)
                    g11 = proj_gen(1, 1)
                    attention(1, 0, drip=g11)
                drain(g11)
              # wpool + xtpool_b0 closed: their SBUF feeds phase-2 tiles
              with tc.tile_pool(name="p2", bufs=1) as p2pool:
                p2_open(p2pool)
                attention(1, 1)
                nc.gpsimd.collective_compute(
                    "AllToAll", mybir.AluOpType.bypass,
                    replica_groups=[list(range(NCORES))],
                    ins=[a2a_in[1][:]], outs=[a2a_out[1][:]],
                )
                # odd-head Z^T loads on the (now idle) scalar ring so
                # they don't queue behind W_O chunk issues on gpsimd
                for j in range(NCORES):
                    nc.scalar.dma_start(p2state["ZTs"][1][:, j, :],
                                        a2a_out[1][j])
                # the whole even-head half runs during the collective,
                # absorbing peer launch skew at this sync point
                p2_load_chunk(1, 0)
                drain(p2half_gen(0))
                drain(p2half_gen(1))

    nc.compile()
    return nc


_CACHE = {}


def _get_nc():
    if "nc" not in _CACHE:
        _CACHE["nc"] = build_nc()
    return _CACHE["nc"]


def make_in_maps(resid_pre, W_Q, W_K, W_V, W_O, b_Q, b_K, b_V, b_O):
    bf = ml_dtypes.bfloat16
    x_bf = np.asarray(resid_pre, np.float32).astype(bf)
    xt = np.ascontiguousarray(x_bf.transpose(0, 2, 1))  # [B, D, S]
    # weights pre-tiled to [H, P, NDT, DH]: w_t[h, p, o, k] = W[h, o*P + p, k]
    def tile_w(W):
        Wb = np.asarray(W, np.float32).astype(bf)
        return np.ascontiguousarray(
            Wb.reshape(H, NDT, P, DH).transpose(0, 2, 1, 3))
    WQ, WK, WV = tile_w(W_Q), tile_w(W_K), tile_w(W_V)
    # wo parity-grouped: wo[p][j] = W_O rows of head 2j+p -> [2, HP, DH, D]
    WOr = np.asarray(W_O, np.float32).reshape(H, DH, D)
    WOp = np.ascontiguousarray(np.stack([WOr[0::2], WOr[1::2]])).astype(bf)
    bQ = np.ascontiguousarray(np.asarray(b_Q, np.float32).T)  # [DH, H]
    bK = np.ascontiguousarray(np.asarray(b_K, np.float32).T)
    bV = np.ascontiguousarray(np.asarray(b_V, np.float32).T)
    bO = np.ascontiguousarray(
        np.asarray(b_O, np.float32)).reshape(1, D).astype(bf)
    in_maps = []
    for c in range(NCORES):
        hs = slice(c * HL, (c + 1) * HL)
        in_maps.append({
            "xt": xt,
            "wq": np.ascontiguousarray(WQ[hs]),
            "wk": np.ascontiguousarray(WK[hs]),
            "wv": np.ascontiguousarray(WV[hs]),
            "bq": np.ascontiguousarray(bQ[:, hs]),
            "bk": np.ascontiguousarray(bK[:, hs]),
            "bv": np.ascontiguousarray(bV[:, hs]),
            "wo": WOp,
            "bo": bO,
        })
    return in_maps


def assemble(results):
    out = np.empty((B, S, D), np.float32)
    for c in range(NCORES):
        b, r = divmod(c, NCORES // B)  # divmod(c, 4)
        out[b, r * QB:(r + 1) * QB] = results[c]["out"]
    return out


def kernel(resid_pre, W_Q, W_K, W_V, W_O, b_Q, b_K, b_V, b_O,
           _trace=False, _return_raw=False):
    nc = _get_nc()
    in_maps = make_in_maps(resid_pre, W_Q, W_K, W_V, W_O, b_Q, b_K, b_V, b_O)
    res = run_bass_kernel_spmd(nc, in_maps, core_ids=list(range(NCORES)),
                               trace=_trace)
    out = assemble(res.results)
    if _return_raw:
        return out, res
    return out


# revision 27
# speedup vs baseline: 1.0478x; 1.0304x over previous
"""Trainium2 (8 NeuronCores) kernel for a dense causal multi-head attention block.

Problem shapes: B=2, S=2048, D=2048, H=16, DH=128 (fp32 in/out).

Distribution strategy (sharding_hint: tensor-parallel over heads):
  Phase 1 (head parallel): core c owns heads {2c, 2c+1}. Per (head, batch)
  section it computes Q^T/K^T/V^T = W^T @ X^T in [DH, S] layout, then causal
  attention fully on-chip:
     scores^T[k, q] = K^T.T @ Q^T          (PE, one matmul per 128x512 tile)
     p = exp(scores / sqrt(DH))            (ACT, straight from PSUM)
     diagonal tiles masked by a 0/1 bf16 mask (DVE)
     z^T[dh, q]  += V_tile.T @ p           (PE, PSUM accumulation over k)
     den_bc[128, q] = ones128.T @ p-sums   (PE rank-reduce; the all-ones lhsT
                                            broadcasts den to all partitions)
     z^T *= reciprocal_approx_fast(den_bc) (DVE custom op; no serial [1,512]
                                            divide, no gpsimd broadcast)
  AllToAll (2 MB bf16) per local head reshards z^T from (head-sharded, all
  rows) to (all heads, 512-row shard).
  Phase 2 (row parallel): out[q, d] = Z^T.T @ W_O + b_O for the core's 512
  rows, split by head parity across the two collectives.

Scheduling: the attention inner loop is exp(ACT)-throughput-bound and the PE
executes strictly in order, so every attention stretch drip-feeds independent
matmuls emitted between its own: sections 1-3 interleave the NEXT section's
projection/transpose matmuls (a generator yielding one PE op at a time);
the last section interleaves the even-head half of the output projection.
DMA queues: sync=X^T loads, scalar=weights/biases, vector=z/out stores,
gpsimd=W_O/Z^T phase-2 loads + collectives.

The host wrapper shards/casts inputs (bf16), runs the SPMD NEFF on cores
0-7, and concatenates the per-core row slices into the full output.
"""

import numpy as np
import ml_dtypes

import concourse.bass as bass
import concourse.mybir as mybir
import concourse.tile as tile
from concourse import bacc
from concourse.bass import ts
from concourse.bass_utils import run_bass_kernel_spmd
from concourse.masks import make_identity

B, S, D, H, DH = 2, 2048, 2048, 16, 128
NCORES = 8
HL = H // NCORES            # heads per core = 2
QB = (B * S) // NCORES      # output rows per core = 512
P = 128
SC = 512                    # free-dim chunk (PSUM bank = 512 fp32)
NSC = S // SC               # 4
NDT = D // P                # 16 contraction tiles for D
NST = S // P                # 16 sequence tiles of 128
NQT = QB // P               # 4 local q tiles in phase 2
NDC = D // SC               # 4 output-dim chunks
HP = H // 2                 # heads per parity group = 8
SCALE = 1.0 / float(np.sqrt(DH))
LOOKAHEAD = 3               # scores tiles in flight ahead of z matmuls
QC_END_PULL = 10            # drip items pulled at each q-chunk boundary

F32 = mybir.dt.float32
BF16 = mybir.dt.bfloat16


def build_nc():
    nc = bacc.Bacc("TRN2", target_bir_lowering=False, debug=False,
                   num_devices=NCORES)

    # xt = X^T per batch ([B, D, S]); weights pre-tiled partition-major on the
    # host. wo is parity-grouped on the host: wo[p][j] = W_O rows of head 2j+p.
    xt = nc.dram_tensor("xt", [B, D, S], BF16, kind="ExternalInput")
    wq = nc.dram_tensor("wq", [HL, P, NDT, DH], BF16, kind="ExternalInput")
    wk = nc.dram_tensor("wk", [HL, P, NDT, DH], BF16, kind="ExternalInput")
    wv = nc.dram_tensor("wv", [HL, P, NDT, DH], BF16, kind="ExternalInput")
    bq = nc.dram_tensor("bq", [DH, HL], F32, kind="ExternalInput")
    bk = nc.dram_tensor("bk", [DH, HL], F32, kind="ExternalInput")
    bv = nc.dram_tensor("bv", [DH, HL], F32, kind="ExternalInput")
    wo = nc.dram_tensor("wo", [2, HP, P, D], BF16, kind="ExternalInput")
    bo = nc.dram_tensor("bo", [1, D], BF16, kind="ExternalInput")
    out = nc.dram_tensor("out", [QB, D], F32, kind="ExternalOutput")

    Exp = mybir.ActivationFunctionType.Exp

    with tile.TileContext(nc) as tc:
        with (
            tc.tile_pool(name="const", bufs=1) as cpool,
            tc.tile_pool(name="dram", bufs=1, space="DRAM") as dpool,
            tc.tile_pool(name="ps_acc", bufs=3, space="PSUM") as ps_acc,
            tc.tile_pool(name="ps_p2", bufs=2, space="PSUM") as ps_p2,
            tc.tile_pool(name="ps_z", bufs=2, space="PSUM") as ps_z,
            tc.tile_pool(name="ps_den", bufs=1, space="PSUM") as ps_den,
        ):
            # ---- biases on the scalar queue (sync queue is for X^T) ----
            bias_sb = {}
            for nm, t in (("q", bq), ("k", bk), ("v", bv)):
                bb = cpool.tile([P, HL], F32, tag=f"b{nm}")
                nc.scalar.dma_start(bb, t.ap())
                bias_sb[nm] = bb
            bo_sb = cpool.tile([1, D], BF16)
            nc.scalar.dma_start(bo_sb, bo.ap())

            # one AllToAll per local head index
            a2a_in = [dpool.tile([NCORES, P, SC], BF16, tag=f"a2a_in{hl}",
                                 name=f"a2a_in{hl}") for hl in range(HL)]
            a2a_out = [dpool.tile([NCORES, P, SC], BF16, tag=f"a2a_out{hl}",
                                  name=f"a2a_out{hl}") for hl in range(HL)]

            with (
                tc.tile_pool(name="qkv", bufs=2) as qkvpool,
                tc.tile_pool(name="small", bufs=4) as spool,
                tc.tile_pool(name="xt1", bufs=1) as xtpool_b1,
            ):
              with tc.tile_pool(name="wpool", bufs=1) as wpool:
                # per-head weight tiles [d_part, d_tile, dh]. The very first
                # weight (wq of head 0) rides the sync ring ahead of X^T so
                # the first matmul can start ~9us in; the rest go on the
                # otherwise-idle gpsimd queue.
                w_sb = []
                for hl in range(HL):
                    per = []
                    for nm, w in (("wq", wq), ("wk", wk), ("wv", wv)):
                        t_sb = wpool.tile([P, NDT, DH], BF16, tag=f"{nm}{hl}")
                        eng = nc.sync if (hl == 0 and nm == "wq") \
                            else nc.gpsimd
                        eng.dma_start(t_sb, w.ap()[hl])
                        per.append(t_sb)
                    w_sb.append(per)

                # gpsimd-built constants, emitted after the weight DMA issues
                ident = cpool.tile([P, P], BF16)
                make_identity(nc, ident)
                ones_sq = cpool.tile([P, P], BF16)
                nc.gpsimd.memset(ones_sq, 1.0)
                # mask[ki, t] = 1.0 iff ki <= t: causal triangle, diag tiles
                mask = cpool.tile([P, P], BF16)
                nc.gpsimd.memset(mask, 1.0)
                nc.gpsimd.affine_select(
                    out=mask, in_=mask, compare_op=mybir.AluOpType.is_ge,
                    fill=0.0, base=0, pattern=[[1, P]], channel_multiplier=-1,
                )

                XT = {}
                QKV = {}

                def proj_gen(hl, b):
                    """Generator emitting the (hl, b) projections and V
                    transposes one PE instruction per yield, so they can be
                    dripped into the previous section's attention bubbles."""
                    QT = qkvpool.tile([P, S], BF16, tag="qt")
                    KT = qkvpool.tile([P, S], BF16, tag="kt")
                    VT = qkvpool.tile([P, S], BF16, tag="vt", bufs=1)
                    V_kd = qkvpool.tile([P, NST, DH], BF16, tag="vkd")
                    QKV[hl, b] = (QT, KT, V_kd)
                    for pi, (dst, bcol) in enumerate((
                        (QT, bias_sb["q"]), (KT, bias_sb["k"]),
                        (VT, bias_sb["v"]),
                    )):
                        wt = w_sb[hl][pi]
                        for sc in range(NSC):
                            ps = ps_p2.tile([P, SC], F32, tag="p2")
                            for dt_ in range(NDT):
                                nc.tensor.matmul(
                                    ps, lhsT=wt[:, dt_, :],
                                    rhs=XT[b][:, dt_, ts(sc, SC)],
                                    start=(dt_ == 0), stop=(dt_ == NDT - 1),
                                    skip_group_check=True)
                                yield
                            # drain + bias on DVE (keeps ACT free for exp)
                            nc.vector.tensor_scalar_add(
                                dst[:, ts(sc, SC)], ps, bcol[:, hl:hl + 1])
                    for st in range(NST):
                        pst = ps_p2.tile([P, P], BF16, tag="p2")
                        nc.tensor.matmul(pst, lhsT=VT[:, ts(st, P)],
                                         rhs=ident, is_transpose=True,
                                         skip_group_check=True)
                        nc.vector.tensor_copy(V_kd[:, st, :], pst)
                        yield

                def proj_first(hl, b):
                    """First section's projections: Q/K/V chains interleaved
                    per s-chunk across 3 PSUM banks so each X^T tile is
                    consumed (3 matmuls) as soon as its DMA lands — the PE
                    tracks the arrival wave instead of stalling per chain."""
                    QT = qkvpool.tile([P, S], BF16, tag="qt")
                    KT = qkvpool.tile([P, S], BF16, tag="kt")
                    VT = qkvpool.tile([P, S], BF16, tag="vt", bufs=1)
                    V_kd = qkvpool.tile([P, NST, DH], BF16, tag="vkd")
                    QKV[hl, b] = (QT, KT, V_kd)
                    dsts = ((QT, bias_sb["q"]), (KT, bias_sb["k"]),
                            (VT, bias_sb["v"]))
                    for sc in range(NSC):
                        banks = [ps_acc.tile([P, SC], F32, tag="acc",
                                             name=f"pf{sc}_{i}")
                                 for i in range(3)]
                        for dt_ in range(NDT):
                            for pi in range(3):
                                nc.tensor.matmul(
                                    banks[pi], lhsT=w_sb[hl][pi][:, dt_, :],
                                    rhs=XT[b][:, dt_, ts(sc, SC)],
                                    start=(dt_ == 0), stop=(dt_ == NDT - 1),
                                    skip_group_check=True)
                        for pi, (dst, bcol) in enumerate(dsts):
                            nc.vector.tensor_scalar_add(
                                dst[:, ts(sc, SC)], banks[pi],
                                bcol[:, hl:hl + 1])
                    for st in range(NST):
                        pst = ps_p2.tile([P, P], BF16, tag="p2")
                        nc.tensor.matmul(pst, lhsT=VT[:, ts(st, P)],
                                         rhs=ident, is_transpose=True,
                                         skip_group_check=True)
                        nc.vector.tensor_copy(V_kd[:, st, :], pst)

                def drain(g):
                    for _ in g:
                        pass

                def pull(g, n):
                    if g is None:
                        return
                    for _ in range(n):
                        if next(g, StopIteration) is StopIteration:
                            return

                def attention(hl, b, drip=None, drip_from=0):
                    """Causal attention for (hl, b); scores pipelined
                    LOOKAHEAD tiles ahead; diagonal tiles at reduced width.
                    One drip item is pulled per kt step, QC_END_PULL per
                    q-chunk boundary, filling PE bubbles left by exp."""
                    QT, KT, V_kd = QKV[hl, b]
                    for qc in range(NSC):
                        dripping = drip if qc >= drip_from else None
                        z_ps = ps_z.tile([P, SC], F32, tag="z")
                        nkt = 4 * qc + 4
                        pexps = {}
                        # exp-sum accumulators: four short bf16 chains keep
                        # the DVE in 2x mode and off the critical path
                        dacc = [spool.tile([P, SC], BF16, tag=f"dac{c}",
                                           bufs=1, name=f"dac{c}")
                                for c in range(4)]

                        def emit_scores(kt, qc=qc, pexps=None, dacc=dacc):
                            j = kt - 4 * qc
                            lo = 128 * j if j >= 0 else 0
                            s_ps = ps_acc.tile([P, SC], F32, tag="acc")
                            nc.tensor.matmul(
                                s_ps[:, :SC - lo], lhsT=KT[:, ts(kt, P)],
                                rhs=QT[:, qc * SC + lo:(qc + 1) * SC],
                                start=True, stop=True)
                            pexp = spool.tile([P, SC], BF16, tag="p", bufs=5)
                            nc.scalar.activation(
                                pexp[:, lo:], s_ps[:, :SC - lo], Exp,
                                bias=0.0, scale=SCALE)
                            if j >= 0:
                                nc.vector.tensor_mul(
                                    pexp[:, lo:lo + P], pexp[:, lo:lo + P],
                                    mask)
                            da = dacc[kt % 4]
                            if kt < 4:
                                nc.vector.tensor_copy(da[:, lo:], pexp[:, lo:])
                            else:
                                nc.vector.tensor_add(
                                    da[:, lo:], da[:, lo:], pexp[:, lo:])
                            pexps[kt] = (pexp, lo)

                        def emit_den(qc=qc, dacc=dacc):
                            # merge chains pairwise on DVE, then one all-ones
                            # matmul both reduces over k AND broadcasts den
                            # across all 128 partitions.
                            clo = [128 * c if qc == 0 else 0 for c in range(4)]
                            nc.vector.tensor_add(
                                dacc[0][:, clo[1]:], dacc[0][:, clo[1]:],
                                dacc[1][:, clo[1]:])
                            nc.vector.tensor_add(
                                dacc[2][:, clo[3]:], dacc[2][:, clo[3]:],
                                dacc[3][:, clo[3]:])
                            nc.vector.tensor_add(
                                dacc[0][:, clo[2]:], dacc[0][:, clo[2]:],
                                dacc[2][:, clo[2]:])
                            den_bc = ps_den.tile([P, SC], F32, tag="den")
                            nc.tensor.matmul(den_bc, lhsT=ones_sq,
                                             rhs=dacc[0], start=True,
                                             stop=True)
                            rb = spool.tile([P, SC], F32, tag="rb", bufs=2)
                            nc.vector.reciprocal_approx_fast(out=rb,
                                                             in_=den_bc)
                            return rb

                        for k0 in range(min(LOOKAHEAD, nkt)):
                            emit_scores(k0, pexps=pexps)
                        rb = None
                        if nkt <= LOOKAHEAD:
                            rb = emit_den()
                        for kt in range(nkt):
                            pull(dripping, 2)
                            if kt + LOOKAHEAD < nkt:
                                emit_scores(kt + LOOKAHEAD, pexps=pexps)
                                if kt + LOOKAHEAD == nkt - 1:
                                    rb = emit_den()
                            pexp, lo = pexps.pop(kt)
                            nc.tensor.matmul(
                                z_ps[:, lo:], lhsT=V_kd[:, kt, :],
                                rhs=pexp[:, lo:],
                                start=(kt == 0), stop=(kt == nkt - 1),
                                skip_group_check=True)
                        zs = spool.tile([P, SC], BF16, tag="zs", bufs=2)
                        nc.vector.tensor_mul(zs, z_ps, rb)
                        nc.scalar.dma_start(a2a_in[hl][4 * b + qc], zs)
                        pull(dripping, QC_END_PULL)

                # ---------- phase-2 helpers ----------
                p2state = {}

                def p2_open(p2pool):
                    # per-parity Z^T tiles: keeps the odd-half DMA writes
                    # (gated on the 2nd collective) from falsely blocking
                    # even-half reads
                    ZTs = [p2pool.tile([P, HP, SC], BF16, tag=f"zt{par}",
                                       name=f"zt{par}") for par in range(2)]
                    bo_b = p2pool.tile([P, D], BF16, tag="bo_b")
                    nc.gpsimd.partition_broadcast(bo_b, bo_sb)
                    parts = {}
                    for qt in range(NQT):
                        for dc in range(NDC):
                            parts[qt, dc] = p2pool.tile(
                                [P, SC], BF16, tag=f"part{qt}_{dc}",
                                name=f"part{qt}_{dc}")
                    p2state.update(ZTs=ZTs, bo_b=bo_b, parts=parts,
                                   pool=p2pool)
                    p2_load_chunk(0, 0, nc.gpsimd)
                    for j in range(NCORES):
                        nc.gpsimd.dma_start(ZTs[0][:, j, :], a2a_out[0][j])

                def p2_load_chunk(par, dc, eng=None):
                    """Stream one 512-col W_O chunk of a parity group;
                    double-buffered per parity so the next chunk prefetches
                    under the current slots."""
                    WOc = p2state["pool"].tile([P, HP, SC], BF16,
                                               tag=f"woc{par}", bufs=2,
                                               name=f"woc{par}_{dc}")
                    p2state["WOc", par, dc] = WOc
                    eng = eng or nc.gpsimd
                    for j in range(HP):
                        eng.dma_start(WOc[:, j, :],
                                      wo.ap()[par][j][:, ts(dc, SC)])

                def p2_slot(par, dc, qt):
                    """Accumulate 8 parity heads into the (qt, dc) output
                    tile; yields per head."""
                    ZT, WOc = p2state["ZTs"][par], p2state["WOc", par, dc]
                    pa = ps_p2.tile([P, SC], F32, tag="p2")
                    for j in range(HP):
                        nc.tensor.matmul(pa, lhsT=ZT[:, j, ts(qt, P)],
                                         rhs=WOc[:, j, :],
                                         start=(j == 0), stop=(j == HP - 1),
                                         skip_group_check=True)
                        yield
                    if par == 0:
                        nc.vector.tensor_add(
                            p2state["parts"][qt, dc], pa,
                            p2state["bo_b"][:, ts(dc, SC)])
                    else:
                        osb = p2state["pool"].tile([P, SC], F32,
                                                   tag="osb", bufs=2)
                        nc.vector.tensor_add(osb, pa,
                                             p2state["parts"][qt, dc])
                        nc.scalar.dma_start(
                            out.ap()[ts(qt, P), ts(dc, SC)], osb)

                def p2half_gen(par):
                    for dc in range(NDC):
                        if dc + 1 < NDC:
                            p2_load_chunk(par, dc + 1)
                        for qt in range(NQT):
                            yield from p2_slot(par, dc, qt)

                # ---------- phase 1 ----------
                with tc.tile_pool(name="xt0", bufs=1) as xtpool_b0:
                    for b in range(B):
                        pool = xtpool_b0 if b == 0 else xtpool_b1
                        xtt = pool.tile([P, NDT, S], BF16, tag=f"xt{b}",
                                        name=f"xt{b}")
                        if b == 0:
                            # s-chunk-major so the first projection chain is
                            # paced by 128KB slices, not 512KB rows
                            for sc in range(NSC):
                                for dt_ in range(NDT):
                                    nc.sync.dma_start(
                                        xtt[:, dt_, ts(sc, SC)],
                                        xt.ap()[b][ts(dt_, P), ts(sc, SC)])
                        else:
                            for dt_ in range(NDT):
                                nc.sync.dma_start(xtt[:, dt_, :],
                                                  xt.ap()[b][ts(dt_, P), :])
                        XT[b] = xtt

                    proj_first(0, 0)
                    g01 = proj_gen(0, 1)
                    attention(0, 0, drip=g01)
                    drain(g01)
                    g10 = proj_gen(1, 0)
                    attention(0, 1, drip=g10)
                    nc.gpsimd.collective_compute(
                        "AllToAll", mybir.AluOpType.bypass,
                        replica_groups=[list(range(NCORES))],
                        ins=[a2a_in[0][:]], outs=[a2a_out[0][:]],
                    )
                    drain(g10)
                    g11 = proj_gen(1, 1)
                    attention(1, 0, drip=g11)
                drain(g11)
              # wpool + xtpool_b0 closed: their SBUF feeds phase-2 tiles
              with tc.tile_pool(name="p2", bufs=1) as p2pool:
                p2_open(p2pool)
                attention(1, 1)
                nc.gpsimd.collective_compute(
                    "AllToAll", mybir.AluOpType.bypass,
                    replica_groups=[list(range(NCORES))],
                    ins=[a2a_in[1][:]], outs=[a2a_out[1][:]],
                )
                # odd-head Z^T loads on the (now idle) scalar ring so
                # they don't queue behind W_O chunk issues on gpsimd
                for j in range(NCORES):
                    nc.scalar.dma_start(p2state["ZTs"][1][:, j, :],
                                        a2a_out[1][j])
                # the whole even-head half runs during the collective,
                # absorbing peer launch skew at this sync point
                p2_load_chunk(1, 0)
                drain(p2half_gen(0))
                # keep-warm pulses: tiny matmuls serialized through DVE
                # drains (~0.5us apart) bridge the collective's peer-skew
                # tail so the PE clock stays at 2.4GHz into the odd half
                for _ in range(16):
                    tp = ps_den.tile([P, SC], F32, tag="den")
                    nc.tensor.matmul(tp[:, :64], lhsT=ones_sq,
                                     rhs=ident[:, :64], start=True,
                                     stop=True, skip_group_check=True)
                    tk = spool.tile([P, 64], BF16, tag="tick", bufs=1)
                    nc.vector.tensor_copy(tk, tp[:, :64])
                drain(p2half_gen(1))

    nc.compile()
    return nc


_CACHE = {}


def _get_nc():
    if "nc" not in _CACHE:
        _CACHE["nc"] = build_nc()
    return _CACHE["nc"]


def make_in_maps(resid_pre, W_Q, W_K, W_V, W_O, b_Q, b_K, b_V, b_O):
    bf = ml_dtypes.bfloat16
    x_bf = np.asarray(resid_pre, np.float32).astype(bf)
    xt = np.ascontiguousarray(x_bf.transpose(0, 2, 1))  # [B, D, S]
    # weights pre-tiled to [H, P, NDT, DH]: w_t[h, p, o, k] = W[h, o*P + p, k]
    def tile_w(W):
        Wb = np.asarray(W, np.float32).astype(bf)
        return np.ascontiguousarray(
            Wb.reshape(H, NDT, P, DH).transpose(0, 2, 1, 3))
    WQ, WK, WV = tile_w(W_Q), tile_w(W_K), tile_w(W_V)
    # wo parity-grouped: wo[p][j] = W_O rows of head 2j+p -> [2, HP, DH, D]
    WOr = np.asarray(W_O, np.float32).reshape(H, DH, D)
    WOp = np.ascontiguousarray(np.stack([WOr[0::2], WOr[1::2]])).astype(bf)
    bQ = np.ascontiguousarray(np.asarray(b_Q, np.float32).T)  # [DH, H]
    bK = np.ascontiguousarray(np.asarray(b_K, np.float32).T)
    bV = np.ascontiguousarray(np.asarray(b_V, np.float32).T)
    bO = np.ascontiguousarray(
        np.asarray(b_O, np.float32)).reshape(1, D).astype(bf)
    in_maps = []
    for c in range(NCORES):
        hs = slice(c * HL, (c + 1) * HL)
        in_maps.append({
            "xt": xt,
            "wq": np.ascontiguousarray(WQ[hs]),
            "wk": np.ascontiguousarray(WK[hs]),
            "wv": np.ascontiguousarray(WV[hs]),
            "bq": np.ascontiguousarray(bQ[:, hs]),
            "bk": np.ascontiguousarray(bK[:, hs]),
            "bv": np.ascontiguousarray(bV[:, hs]),
            "wo": WOp,
            "bo": bO,
        })
    return in_maps


def assemble(results):
    out = np.empty((B, S, D), np.float32)
    for c in range(NCORES):
        b, r = divmod(c, NCORES // B)  # divmod(c, 4)
        out[b, r * QB:(r + 1) * QB] = results[c]["out"]
    return out


def kernel(resid_pre, W_Q, W_K, W_V, W_O, b_Q, b_K, b_V, b_O,
           _trace=False, _return_raw=False):
    nc = _get_nc()
    in_maps = make_in_maps(resid_pre, W_Q, W_K, W_V, W_O, b_Q, b_K, b_V, b_O)
    res = run_bass_kernel_spmd(nc, in_maps, core_ids=list(range(NCORES)),
                               trace=_trace)
    out = assemble(res.results)
    if _return_raw:
        return out, res
    return out
